# revision 15
# baseline (speedup 1.0000x reference)
"""Trainium2 Bass kernel for NeuralODEMemory (nn_NeuralODEMemory_28355374088720).

Math (reference):
    dt = 0.1, 10 Euler steps over h (N=65536 rows, D=512):
        z = [h, t]                              # time feature column
        deriv = tanh(tanh(z @ W1.T + b1) @ W2.T + b2)
        h <- h + dt * deriv
    gate  = sigmoid([x, h10] @ Wg.T + bg)
    out   = gate * h10 + (1 - gate) * x

Kernel strategy (v2 — fp8 ODE loop, bf16 gate):
  * Data-parallel over 8 NeuronCores (8192 rows each); weights replicated.
  * Feature-major ("transposed") activation layout [D, rows] on chip: weights
    are the stationary operand, activations stream, each layer's PSUM output
    is already in the layout the next layer consumes.
  * ODE-loop matmuls (L1, L2) run in float8-e4m3 with DoubleRow (K=256/pass,
    ~1.77x the bf16 MM stream).  Numerical design validated by host-side
    simulation (rel err ~7e-3 vs the 2e-2 gate):
      - h accumulator stays fp32 (bf16 h pins max-err at 2.6e-2).
      - gate matmul stays bf16 (fp8 gate is the dominant max-err term:
        2.4e-2 alone; bf16 gate keeps the whole pipeline under 1e-2).
      - per-step *dither*: NPHASE fp8 copies of W1/W2 scaled by (1+eps_p);
        the ACT input scale undoes (1+eps_p) exactly.  This decorrelates
        the step-to-step weight quantization error (fixed e4m3 weight error
        otherwise integrates linearly through the 10 Euler steps).
      - the fp8 h-mirror is refreshed only every MP steps: the mirror lag
        (<= dt*|deriv| = 0.1) is below e4m3 quantization noise of h, and it
        halves the DVE cast load.
      - TRN e4m3 max normal is +-240 (not OCP's 448): all fp8 casts are
        host-side and clipped to +-240; on-chip casts only see tanh outputs
        in [-1,1] and h (|h|<7).
  * The time-feature column is folded into a per-step bias:
    z @ W1.T = h @ W1[:, :D].T + t * W1[:, D], so b1_eff(s) = b1 + t_s*W1[:,D].
    Bias-add and the fp8 scale-undo ride the ACT instruction (f(in*s + b)).
  * RBLK=2048 rows per block: PSUM tiles are [128, 2048] (4 banks, 2 bufs =
    all 8 banks) so each ACT instruction processes N=2048 — the ~460-cycle
    per-instruction ACT overhead made ACT the post-fp8 bottleneck at N=1024.
  * All elementwise work stays on DVE (GpSimd shares its SBUF port with
    DVE's 2-port perf mode — splitting buys little): h update is one fp32
    scalar_tensor_tensor per chunk, mirrors are plain casts, the gate
    combine runs at 16-bit 2x rate.
  * Host does the cheap prep: weight transpose/scale/cast, x/h transposes.
"""

import os
from contextlib import ExitStack

import numpy as np
import ml_dtypes

N_TOTAL = 65536
D = 512
NCORES = 8
NPC = N_TOTAL // NCORES          # rows per core
NUM_STEPS = 10
TIME_INTERVAL = 1.0
DT = TIME_INTERVAL / NUM_STEPS
P = 128
FK = D // P                      # feature chunks of 128 (4)
MMN = 512                        # matmul free dim (one PSUM bank)
RBLK = 2048                      # rows per block (4-bank PSUM groups)
NSUB = RBLK // MMN

NPHASE = 3                       # dithered fp8 weight copies
MP = 2                           # h-mirror refresh period (steps)
AW1 = 2.0 ** 12                  # fp8 scale on W1 (|W1|<=0.0442 -> <=189)
AW2 = 2.0 ** 12                  # fp8 scale on W2
# golden-ratio multiplicative dither phases, undone exactly in the ACT scale
EPS = [0.09 * ((p * 0.6180339887) % 1.0) - 0.045 for p in range(NPHASE)]

_CACHE = {}
LAST = {}                        # stash of the last run's BassKernelResults


def _build(npc):
    import concourse.mybir as mybir
    import concourse.tile as tile
    from concourse import bacc

    f32 = mybir.dt.float32
    bf16 = mybir.dt.bfloat16
    fp8 = mybir.dt.float8e4
    Act = mybir.ActivationFunctionType
    Alu = mybir.AluOpType
    DR = mybir.MatmulPerfMode.DoubleRow

    nblk = npc // RBLK

    nc = bacc.Bacc("TRN2", target_bir_lowering=False, debug=False,
                   num_devices=NCORES)

    hT = nc.dram_tensor("hT", [D, npc], f32, kind="ExternalInput").ap()
    xTb = nc.dram_tensor("xTb", [D, npc], bf16, kind="ExternalInput").ap()
    w1t = nc.dram_tensor("w1t", [NPHASE * D, D], fp8, kind="ExternalInput").ap()
    w2t = nc.dram_tensor("w2t", [NPHASE * D, D], fp8, kind="ExternalInput").ap()
    wgt = nc.dram_tensor("wgt", [2 * D, D], bf16, kind="ExternalInput").ap()
    b1e = nc.dram_tensor("b1e", [P, NUM_STEPS * FK], f32, kind="ExternalInput").ap()
    b2c = nc.dram_tensor("b2c", [P, FK], f32, kind="ExternalInput").ap()
    bgc = nc.dram_tensor("bgc", [P, FK], f32, kind="ExternalInput").ap()
    outT = nc.dram_tensor("outT", [D, npc], bf16, kind="ExternalOutput").ap()

    hTr = hT.rearrange("(k p) r -> p k r", p=P)
    xTr = xTb.rearrange("(k p) r -> p k r", p=P)
    outTr = outT.rearrange("(k p) r -> p k r", p=P)

    with tile.TileContext(nc) as tc:
        with ExitStack() as ctx:
            consts = ctx.enter_context(tc.tile_pool(name="consts", bufs=1))
            hpool = ctx.enter_context(tc.tile_pool(name="h", bufs=2))
            hbp = ctx.enter_context(tc.tile_pool(name="hb", bufs=2))
            apool = ctx.enter_context(tc.tile_pool(name="a", bufs=2))
            dpool = ctx.enter_context(tc.tile_pool(name="d", bufs=2))
            xbp = ctx.enter_context(tc.tile_pool(name="xb", bufs=1))
            hgp = ctx.enter_context(tc.tile_pool(name="hg", bufs=1))
            gp = ctx.enter_context(tc.tile_pool(name="g", bufs=2))
            scp = ctx.enter_context(tc.tile_pool(name="sc", bufs=2))
            otp = ctx.enter_context(tc.tile_pool(name="ot", bufs=2))
            psp = ctx.enter_context(tc.tile_pool(name="ps", bufs=2, space="PSUM"))

            # Replicated constants, resident for the whole kernel.
            w1 = consts.tile([P, NPHASE * FK, D], fp8)
            w2 = consts.tile([P, NPHASE * FK, D], fp8)
            wg = consts.tile([P, 2 * FK, D], bf16)
            b1 = consts.tile([P, NUM_STEPS * FK], f32)
            b2 = consts.tile([P, FK], f32)
            bg = consts.tile([P, FK], f32)

            def load_block(blk):
                rs = blk * RBLK
                h = hpool.tile([P, FK, RBLK], f32, tag="h")
                for k in range(FK):
                    nc.sync.dma_start(h[:, k], hTr[:, k, rs:rs + RBLK])
                xb = xbp.tile([P, FK, RBLK], bf16, tag="xb")
                nc.sync.dma_start(xb[:], xTr[:, :, rs:rs + RBLK])
                return h, xb

            # block-0 activations are queued BEFORE the weight download so
            # the first matmul only waits for h chunk 0 + w1 phase 0
            h_cur, xb_cur = load_block(0)

            # per-phase weight loads: the first L1 matmul needs only phase 0
            w1r = w1t.rearrange("(f p) m -> p f m", p=P)
            w2r = w2t.rearrange("(f p) m -> p f m", p=P)
            for ph in range(NPHASE):
                nc.sync.dma_start(w1[:, ph * FK:(ph + 1) * FK],
                                  w1r[:, ph * FK:(ph + 1) * FK])
                nc.sync.dma_start(w2[:, ph * FK:(ph + 1) * FK],
                                  w2r[:, ph * FK:(ph + 1) * FK])
            nc.sync.dma_start(wg[:], wgt.rearrange("(k p) m -> p k m", p=P))
            nc.sync.dma_start(b1[:], b1e)
            nc.sync.dma_start(b2[:], b2c)
            nc.sync.dma_start(bg[:], bgc)

            def mm_dr(ps_t, wtile, base, m, rhs):
                # one DR accumulation group: 2 k-pair groups x NSUB banks
                for kk in range(FK // 2):
                    for sub in range(NSUB):
                        nc.tensor.matmul(
                            ps_t[:, sub * MMN:(sub + 1) * MMN],
                            wtile[:, base + 2 * kk:base + 2 * kk + 2,
                                  m * P:(m + 1) * P],
                            rhs[:, 2 * kk:2 * kk + 2,
                                sub * MMN:(sub + 1) * MMN],
                            start=kk == 0, stop=kk == FK // 2 - 1,
                            perf_mode=DR)

            def mm_bf(ps_t, wk, m, rhs, rk, start, stop):
                for sub in range(NSUB):
                    nc.tensor.matmul(
                        ps_t[:, sub * MMN:(sub + 1) * MMN],
                        wg[:, wk, m * P:(m + 1) * P],
                        rhs[:, rk, sub * MMN:(sub + 1) * MMN],
                        start=start, stop=stop)

            def gate_group(pend, m):
                # one gate output-chunk of an earlier block: 32 bf16 MMs +
                # sigmoid + combine + store.  Interleaved into a later
                # block's step stream, this fills the PE slack left by the
                # ACT-paced layer pipeline instead of serializing at the
                # block boundary.
                hg_p, xb_p, rs_p = pend
                ps = psp.tile([P, RBLK], f32, tag="ps")
                for k in range(FK):
                    mm_bf(ps, k, m, xb_p, k, k == 0, False)
                for k in range(FK):
                    mm_bf(ps, FK + k, m, hg_p, k, False, k == FK - 1)
                g = gp.tile([P, RBLK], bf16, tag="g")
                nc.scalar.activation(g[:], ps[:], Act.Sigmoid,
                                     bias=bg[:, m:m + 1], scale=1.0)
                # out = x + g * (h10 - x), all 16-bit 2x-rate ops
                dif = scp.tile([P, RBLK], bf16, tag="sc")
                nc.vector.tensor_tensor(dif[:], hg_p[:, m], xb_p[:, m],
                                        Alu.subtract)
                nc.vector.tensor_tensor(dif[:], g[:], dif[:], Alu.mult)
                ot = otp.tile([P, RBLK], bf16, tag="ot")
                nc.vector.tensor_tensor(ot[:], xb_p[:, m], dif[:], Alu.add)
                nc.sync.dma_start(outTr[:, m, rs_p:rs_p + RBLK], ot[:])

            pending = None
            for blk in range(nblk):
                rs = blk * RBLK
                h, xb = h_cur, xb_cur

                # mirror for step 0 (exact)
                hb = hbp.tile([P, FK, RBLK], fp8, tag="hb")
                for k in range(FK):
                    nc.vector.tensor_copy(hb[:, k], h[:, k])
                if blk + 1 < nblk:
                    h_cur, xb_cur = load_block(blk + 1)   # prefetch
                hg = None
                for s in range(NUM_STEPS):
                    ph = s % NPHASE
                    s1 = 1.0 / (AW1 * (1.0 + EPS[ph]))
                    s2 = 1.0 / (AW2 * (1.0 + EPS[ph]))
                    # layer 1: a = tanh(W1p.T-chunks @ hb + b1_eff(s))
                    a = apool.tile([P, FK, RBLK], fp8, tag="a")
                    for m in range(FK):
                        ps = psp.tile([P, RBLK], f32, tag="ps")
                        mm_dr(ps, w1, ph * FK, m, hb)
                        col = s * FK + m
                        nc.scalar.activation(a[:, m], ps[:], Act.Tanh,
                                             bias=b1[:, col:col + 1], scale=s1)
                    # layer 2: d = tanh(W2p.T-chunks @ a + b2)
                    d = dpool.tile([P, FK, RBLK], bf16, tag="d")
                    for m in range(FK):
                        ps = psp.tile([P, RBLK], f32, tag="ps")
                        mm_dr(ps, w2, ph * FK, m, a)
                        nc.scalar.activation(d[:, m], ps[:], Act.Tanh,
                                             bias=b2[:, m:m + 1], scale=s2)
                    if pending is not None and s in (2, 4, 6, 8):
                        gate_group(pending, (s - 2) // 2)
                    if s < NUM_STEPS - 1:
                        # next step's mirror, cast BEFORE the update (lag-1):
                        # keeps the DVE STT/cast chain off the PE critical
                        # path (a lag-0 refresh stalls PE ~7.5us and lets HAM
                        # re-throttle).  The 1-step-stale mirror error
                        # (<= dt*|deriv|) is below e4m3 noise (sim: 1.07e-2).
                        hb = hbp.tile([P, FK, RBLK], fp8, tag="hb")
                        for k in range(FK):
                            nc.vector.tensor_copy(hb[:, k], h[:, k])
                        # h += dt * d  (fp32, in place)
                        for k in range(FK):
                            nc.vector.scalar_tensor_tensor(
                                h[:, k], d[:, k], float(DT), h[:, k],
                                Alu.mult, Alu.add)
                    else:
                        # final step: the update writes the bf16 gate operand
                        # directly (h10 is never needed in fp32), so gate
                        # h-side k-group k starts as soon as chunk k lands
                        hg = hgp.tile([P, FK, RBLK], bf16, tag="hg")
                        for k in range(FK):
                            nc.vector.scalar_tensor_tensor(
                                hg[:, k], d[:, k], float(DT), h[:, k],
                                Alu.mult, Alu.add)

                # this block's gate is deferred into the next block's steps
                pending = (hg, xb, rs)

            # drain the last block's gate
            for m in range(FK):
                gate_group(pending, m)

    nc.compile()
    return nc


def _get_nc(npc):
    if npc not in _CACHE:
        _CACHE[npc] = _build(npc)
    return _CACHE[npc]


def _fp8_np():
    import concourse.mybir as mybir
    return mybir.dt.np(mybir.dt.float8e4)


def _host_prep(W1, b1, W2, b2, Wg, bg):
    E4 = _fp8_np()
    W1 = np.asarray(W1, np.float32)
    W2 = np.asarray(W2, np.float32)
    Wg = np.asarray(Wg, np.float32)
    b1 = np.asarray(b1, np.float32)
    b2 = np.asarray(b2, np.float32)
    bg = np.asarray(bg, np.float32)

    # NPHASE dithered fp8 copies, [in, out] layout, clipped to TRN e4m3 range
    w1t = np.concatenate(
        [np.clip(AW1 * (1.0 + e) * W1[:, :D], -240, 240).T for e in EPS],
        axis=0)
    w2t = np.concatenate(
        [np.clip(AW2 * (1.0 + e) * W2, -240, 240).T for e in EPS], axis=0)
    w1t = np.ascontiguousarray(w1t).astype(E4)
    w2t = np.ascontiguousarray(w2t).astype(E4)
    wgt = np.ascontiguousarray(
        np.concatenate([Wg[:, :D].T, Wg[:, D:].T], axis=0)
    ).astype(ml_dtypes.bfloat16)

    ts = (DT * np.arange(NUM_STEPS)).astype(np.float32)
    b1r = b1.reshape(FK, P)                                        # [m, p]
    wtr = np.ascontiguousarray(W1[:, D]).reshape(FK, P)            # [m, p]
    b1e = b1r[None, :, :] + ts[:, None, None] * wtr[None, :, :]    # [s, m, p]
    b1e = np.ascontiguousarray(b1e.transpose(2, 0, 1).reshape(P, NUM_STEPS * FK))
    b2c = np.ascontiguousarray(b2.reshape(FK, P).T)
    bgc = np.ascontiguousarray(bg.reshape(FK, P).T)
    return dict(w1t=w1t, w2t=w2t, wgt=wgt,
                b1e=b1e.astype(np.float32),
                b2c=b2c.astype(np.float32), bgc=bgc.astype(np.float32))


def kernel(current_node_features, previous_hidden_state, W1, b1, W2, b2, Wg, bg):
    from concourse.bass_utils import run_bass_kernel_spmd

    x = np.asarray(current_node_features, np.float32)
    h0 = np.asarray(previous_hidden_state, np.float32)
    weights = _host_prep(W1, b1, W2, b2, Wg, bg)

    in_maps = []
    for c in range(NCORES):
        sl = slice(c * NPC, (c + 1) * NPC)
        in_maps.append(dict(
            hT=np.ascontiguousarray(h0[sl].T),
            xTb=np.ascontiguousarray(x[sl].T).astype(ml_dtypes.bfloat16),
            **weights,
        ))

    nc = _get_nc(NPC)
    trace = bool(os.environ.get("BASS_TRACE"))
    if trace:
        try:
            import antenv.axon_hooks  # noqa: F401
        except ImportError:
            # no NTFF shim installed (see test.py) -> tracing would crash
            os.environ["BASS_NEVER_TRACE"] = "1"
            trace = False
    res = run_bass_kernel_spmd(nc, in_maps, core_ids=list(range(NCORES)),
                               trace=trace)
    LAST["res"] = res

    out = np.empty((N_TOTAL, D), np.float32)
    for c in range(NCORES):
        out[c * NPC:(c + 1) * NPC] = res.results[c]["outT"].T.astype(np.float32)
    return out, out


# revision 19
# speedup vs baseline: 1.0658x; 1.0658x over previous
"""Trainium2 Bass kernel for NeuralODEMemory (nn_NeuralODEMemory_28355374088720).

Math (reference):
    dt = 0.1, 10 Euler steps over h (N=65536 rows, D=512):
        z = [h, t]                              # time feature column
        deriv = tanh(tanh(z @ W1.T + b1) @ W2.T + b2)
        h <- h + dt * deriv
    gate  = sigmoid([x, h10] @ Wg.T + bg)
    out   = gate * h10 + (1 - gate) * x

Kernel strategy (v2 — fp8 ODE loop, bf16 gate):
  * Data-parallel over 8 NeuronCores (8192 rows each); weights replicated.
  * Feature-major ("transposed") activation layout [D, rows] on chip: weights
    are the stationary operand, activations stream, each layer's PSUM output
    is already in the layout the next layer consumes.
  * ODE-loop matmuls (L1, L2) run in float8-e4m3 with DoubleRow (K=256/pass,
    ~1.77x the bf16 MM stream).  Numerical design validated by host-side
    simulation (rel err ~7e-3 vs the 2e-2 gate):
      - h accumulator stays fp32 (bf16 h pins max-err at 2.6e-2).
      - gate matmul stays bf16 (fp8 gate is the dominant max-err term:
        2.4e-2 alone; bf16 gate keeps the whole pipeline under 1e-2).
      - per-step *dither*: NPHASE fp8 copies of W1/W2 scaled by (1+eps_p);
        the ACT input scale undoes (1+eps_p) exactly.  This decorrelates
        the step-to-step weight quantization error (fixed e4m3 weight error
        otherwise integrates linearly through the 10 Euler steps).
      - the fp8 h-mirror is refreshed only every MP steps: the mirror lag
        (<= dt*|deriv| = 0.1) is below e4m3 quantization noise of h, and it
        halves the DVE cast load.
      - TRN e4m3 max normal is +-240 (not OCP's 448): all fp8 casts are
        host-side and clipped to +-240; on-chip casts only see tanh outputs
        in [-1,1] and h (|h|<7).
  * The time-feature column is folded into a per-step bias:
    z @ W1.T = h @ W1[:, :D].T + t * W1[:, D], so b1_eff(s) = b1 + t_s*W1[:,D].
    Bias-add and the fp8 scale-undo ride the ACT instruction (f(in*s + b)).
  * RBLK=2048 rows per block: PSUM tiles are [128, 2048] (4 banks, 2 bufs =
    all 8 banks) so each ACT instruction processes N=2048 — the ~460-cycle
    per-instruction ACT overhead made ACT the post-fp8 bottleneck at N=1024.
  * All elementwise work stays on DVE (GpSimd shares its SBUF port with
    DVE's 2-port perf mode — splitting buys little): h update is one fp32
    scalar_tensor_tensor per chunk, mirrors are plain casts, the gate
    combine runs at 16-bit 2x rate.
  * Host does the cheap prep: weight transpose/scale/cast, x/h transposes.
"""

import os
from contextlib import ExitStack

import numpy as np
import ml_dtypes

N_TOTAL = 65536
D = 512
NCORES = 8
NPC = N_TOTAL // NCORES          # rows per core
NUM_STEPS = 10
TIME_INTERVAL = 1.0
DT = TIME_INTERVAL / NUM_STEPS
P = 128
FK = D // P                      # feature chunks of 128 (4)
MMN = 512                        # matmul free dim (one PSUM bank)
RBLK = 2048                      # rows per block (4-bank PSUM groups)
NSUB = RBLK // MMN

NPHASE = 3                       # dithered fp8 weight copies
MP = 2                           # h-mirror refresh period (steps)
AW1 = 2.0 ** 12                  # fp8 scale on W1 (|W1|<=0.0442 -> <=189)
AW2 = 2.0 ** 12                  # fp8 scale on W2
# golden-ratio multiplicative dither phases, undone exactly in the ACT scale
EPS = [0.09 * ((p * 0.6180339887) % 1.0) - 0.045 for p in range(NPHASE)]

_CACHE = {}
LAST = {}                        # stash of the last run's BassKernelResults


def _build(npc):
    import concourse.mybir as mybir
    import concourse.tile as tile
    from concourse import bacc

    f32 = mybir.dt.float32
    bf16 = mybir.dt.bfloat16
    fp8 = mybir.dt.float8e4
    Act = mybir.ActivationFunctionType
    Alu = mybir.AluOpType
    DR = mybir.MatmulPerfMode.DoubleRow

    nblk = npc // RBLK

    nc = bacc.Bacc("TRN2", target_bir_lowering=False, debug=False,
                   num_devices=NCORES)

    hT = nc.dram_tensor("hT", [D, npc], f32, kind="ExternalInput").ap()
    xTb = nc.dram_tensor("xTb", [D, npc], bf16, kind="ExternalInput").ap()
    w1t = nc.dram_tensor("w1t", [NPHASE * D, D], fp8, kind="ExternalInput").ap()
    w2t = nc.dram_tensor("w2t", [NPHASE * D, D], fp8, kind="ExternalInput").ap()
    wgt = nc.dram_tensor("wgt", [2 * D, D], bf16, kind="ExternalInput").ap()
    b1e = nc.dram_tensor("b1e", [P, NUM_STEPS * FK], f32, kind="ExternalInput").ap()
    b2c = nc.dram_tensor("b2c", [P, FK], f32, kind="ExternalInput").ap()
    bgc = nc.dram_tensor("bgc", [P, FK], f32, kind="ExternalInput").ap()
    outT = nc.dram_tensor("outT", [D, npc], bf16, kind="ExternalOutput").ap()

    hTr = hT.rearrange("(k p) r -> p k r", p=P)
    xTr = xTb.rearrange("(k p) r -> p k r", p=P)
    outTr = outT.rearrange("(k p) r -> p k r", p=P)

    with tile.TileContext(nc) as tc:
        with ExitStack() as ctx:
            consts = ctx.enter_context(tc.tile_pool(name="consts", bufs=1))
            hpool = ctx.enter_context(tc.tile_pool(name="h", bufs=2))
            hbp = ctx.enter_context(tc.tile_pool(name="hb", bufs=2))
            apool = ctx.enter_context(tc.tile_pool(name="a", bufs=2))
            dpool = ctx.enter_context(tc.tile_pool(name="d", bufs=2))
            xbp = ctx.enter_context(tc.tile_pool(name="xb", bufs=1))
            hgp = ctx.enter_context(tc.tile_pool(name="hg", bufs=1))
            gp = ctx.enter_context(tc.tile_pool(name="g", bufs=2))
            scp = ctx.enter_context(tc.tile_pool(name="sc", bufs=2))
            otp = ctx.enter_context(tc.tile_pool(name="ot", bufs=2))
            psp = ctx.enter_context(tc.tile_pool(name="ps", bufs=2, space="PSUM"))

            # Replicated constants, resident for the whole kernel.
            w1 = consts.tile([P, NPHASE * FK, D], fp8)
            w2 = consts.tile([P, NPHASE * FK, D], fp8)
            wg = consts.tile([P, 2 * FK, D], bf16)
            b1 = consts.tile([P, NUM_STEPS * FK], f32)
            b2 = consts.tile([P, FK], f32)
            bg = consts.tile([P, FK], f32)

            def load_block(blk):
                rs = blk * RBLK
                h = hpool.tile([P, FK, RBLK], f32, tag="h")
                # per-chunk loads so the first mirror starts as soon as its
                # chunk lands
                for k in range(FK):
                    nc.sync.dma_start(h[:, k], hTr[:, k, rs:rs + RBLK])
                xb = xbp.tile([P, FK, RBLK], bf16, tag="xb")
                nc.sync.dma_start(xb[:], xTr[:, :, rs:rs + RBLK])
                return h, xb

            # block-0 activations are queued BEFORE the weight download so
            # the first matmul only waits for h chunk 0 + w1 phase 0
            h_cur, xb_cur = load_block(0)

            # per-phase weight loads: the first L1 matmul needs only phase 0
            w1r = w1t.rearrange("(f p) m -> p f m", p=P)
            w2r = w2t.rearrange("(f p) m -> p f m", p=P)
            for ph in range(NPHASE):
                nc.sync.dma_start(w1[:, ph * FK:(ph + 1) * FK],
                                  w1r[:, ph * FK:(ph + 1) * FK])
                nc.sync.dma_start(w2[:, ph * FK:(ph + 1) * FK],
                                  w2r[:, ph * FK:(ph + 1) * FK])
            nc.sync.dma_start(wg[:], wgt.rearrange("(k p) m -> p k m", p=P))
            nc.sync.dma_start(b1[:], b1e)
            nc.sync.dma_start(b2[:], b2c)
            nc.sync.dma_start(bg[:], bgc)

            def mm_dr(ps_t, wtile, base, m, rhs):
                # one DR accumulation group: 2 k-pair groups x NSUB banks
                for kk in range(FK // 2):
                    for sub in range(NSUB):
                        nc.tensor.matmul(
                            ps_t[:, sub * MMN:(sub + 1) * MMN],
                            wtile[:, base + 2 * kk:base + 2 * kk + 2,
                                  m * P:(m + 1) * P],
                            rhs[:, 2 * kk:2 * kk + 2,
                                sub * MMN:(sub + 1) * MMN],
                            start=kk == 0, stop=kk == FK // 2 - 1,
                            perf_mode=DR)

            def mm_bf(ps_t, wk, m, rhs, rk, start, stop):
                for sub in range(NSUB):
                    nc.tensor.matmul(
                        ps_t[:, sub * MMN:(sub + 1) * MMN],
                        wg[:, wk, m * P:(m + 1) * P],
                        rhs[:, rk, sub * MMN:(sub + 1) * MMN],
                        start=start, stop=stop)

            for blk in range(nblk):
                rs = blk * RBLK
                h, xb = h_cur, xb_cur

                # mirror for step 0 (exact)
                hb = hbp.tile([P, FK, RBLK], fp8, tag="hb")
                for k in range(FK):
                    nc.vector.tensor_copy(hb[:, k], h[:, k])
                if blk + 1 < nblk:
                    h_cur, xb_cur = load_block(blk + 1)   # prefetch
                hg = None
                for s in range(NUM_STEPS):
                    ph = s % NPHASE
                    s1 = 1.0 / (AW1 * (1.0 + EPS[ph]))
                    s2 = 1.0 / (AW2 * (1.0 + EPS[ph]))
                    # layer 1: a = tanh(W1p.T-chunks @ hb + b1_eff(s))
                    a = apool.tile([P, FK, RBLK], fp8, tag="a")
                    for m in range(FK):
                        ps = psp.tile([P, RBLK], f32, tag="ps")
                        mm_dr(ps, w1, ph * FK, m, hb)
                        col = s * FK + m
                        nc.scalar.activation(a[:, m], ps[:], Act.Tanh,
                                             bias=b1[:, col:col + 1], scale=s1)
                    # layer 2: d = tanh(W2p.T-chunks @ a + b2)
                    d = dpool.tile([P, FK, RBLK], bf16, tag="d")
                    for m in range(FK):
                        ps = psp.tile([P, RBLK], f32, tag="ps")
                        mm_dr(ps, w2, ph * FK, m, a)
                        nc.scalar.activation(d[:, m], ps[:], Act.Tanh,
                                             bias=b2[:, m:m + 1], scale=s2)
                    if s < NUM_STEPS - 1:
                        # next step's mirror, cast BEFORE the update (lag-1):
                        # keeps the DVE STT/cast chain off the PE critical
                        # path (a lag-0 refresh stalls PE ~7.5us and lets HAM
                        # re-throttle).  The 1-step-stale mirror error
                        # (<= dt*|deriv|) is below e4m3 noise (sim: 1.07e-2).
                        hb = hbp.tile([P, FK, RBLK], fp8, tag="hb")
                        for k in range(FK):
                            nc.vector.tensor_copy(hb[:, k], h[:, k])
                        # h += dt * d  (fp32, in place)
                        for k in range(FK):
                            nc.vector.scalar_tensor_tensor(
                                h[:, k], d[:, k], float(DT), h[:, k],
                                Alu.mult, Alu.add)
                    else:
                        # final step: the update writes the bf16 gate operand
                        # directly (h10 is never needed in fp32), so gate
                        # h-side k-group k starts as soon as chunk k lands
                        hg = hgp.tile([P, FK, RBLK], bf16, tag="hg")
                        for k in range(FK):
                            nc.vector.scalar_tensor_tensor(
                                hg[:, k], d[:, k], float(DT), h[:, k],
                                Alu.mult, Alu.add)

                # gate (bf16): g = sigmoid([x, h10] @ Wg.T + bg)
                # h-side k-groups first, in STT completion order, so the
                # gate matmuls start before the final update chain finishes
                for m in range(FK):
                    ps = psp.tile([P, RBLK], f32, tag="ps")
                    for k in range(FK):
                        mm_bf(ps, FK + k, m, hg, k, k == 0, False)
                    for k in range(FK):
                        mm_bf(ps, k, m, xb, k, False, k == FK - 1)
                    g = gp.tile([P, RBLK], bf16, tag="g")
                    nc.scalar.activation(g[:], ps[:], Act.Sigmoid,
                                         bias=bg[:, m:m + 1], scale=1.0)
                    # out = x + g * (h10 - x), all 16-bit 2x-rate ops
                    dif = scp.tile([P, RBLK], bf16, tag="sc")
                    nc.vector.tensor_tensor(dif[:], hg[:, m], xb[:, m],
                                            Alu.subtract)
                    nc.vector.tensor_tensor(dif[:], g[:], dif[:], Alu.mult)
                    ot = otp.tile([P, RBLK], bf16, tag="ot")
                    nc.vector.tensor_tensor(ot[:], xb[:, m], dif[:], Alu.add)
                    nc.sync.dma_start(outTr[:, m, rs:rs + RBLK], ot[:])

    nc.compile()
    return nc


def _get_nc(npc):
    if npc not in _CACHE:
        _CACHE[npc] = _build(npc)
    return _CACHE[npc]


def _fp8_np():
    import concourse.mybir as mybir
    return mybir.dt.np(mybir.dt.float8e4)


def _host_prep(W1, b1, W2, b2, Wg, bg):
    E4 = _fp8_np()
    W1 = np.asarray(W1, np.float32)
    W2 = np.asarray(W2, np.float32)
    Wg = np.asarray(Wg, np.float32)
    b1 = np.asarray(b1, np.float32)
    b2 = np.asarray(b2, np.float32)
    bg = np.asarray(bg, np.float32)

    # NPHASE dithered fp8 copies, [in, out] layout, clipped to TRN e4m3 range
    w1t = np.concatenate(
        [np.clip(AW1 * (1.0 + e) * W1[:, :D], -240, 240).T for e in EPS],
        axis=0)
    w2t = np.concatenate(
        [np.clip(AW2 * (1.0 + e) * W2, -240, 240).T for e in EPS], axis=0)
    w1t = np.ascontiguousarray(w1t).astype(E4)
    w2t = np.ascontiguousarray(w2t).astype(E4)
    wgt = np.ascontiguousarray(
        np.concatenate([Wg[:, :D].T, Wg[:, D:].T], axis=0)
    ).astype(ml_dtypes.bfloat16)

    ts = (DT * np.arange(NUM_STEPS)).astype(np.float32)
    b1r = b1.reshape(FK, P)                                        # [m, p]
    wtr = np.ascontiguousarray(W1[:, D]).reshape(FK, P)            # [m, p]
    b1e = b1r[None, :, :] + ts[:, None, None] * wtr[None, :, :]    # [s, m, p]
    b1e = np.ascontiguousarray(b1e.transpose(2, 0, 1).reshape(P, NUM_STEPS * FK))
    b2c = np.ascontiguousarray(b2.reshape(FK, P).T)
    bgc = np.ascontiguousarray(bg.reshape(FK, P).T)
    return dict(w1t=w1t, w2t=w2t, wgt=wgt,
                b1e=b1e.astype(np.float32),
                b2c=b2c.astype(np.float32), bgc=bgc.astype(np.float32))


def kernel(current_node_features, previous_hidden_state, W1, b1, W2, b2, Wg, bg):
    from concourse.bass_utils import run_bass_kernel_spmd

    x = np.asarray(current_node_features, np.float32)
    h0 = np.asarray(previous_hidden_state, np.float32)
    weights = _host_prep(W1, b1, W2, b2, Wg, bg)

    in_maps = []
    for c in range(NCORES):
        sl = slice(c * NPC, (c + 1) * NPC)
        in_maps.append(dict(
            hT=np.ascontiguousarray(h0[sl].T),
            xTb=np.ascontiguousarray(x[sl].T).astype(ml_dtypes.bfloat16),
            **weights,
        ))

    nc = _get_nc(NPC)
    trace = bool(os.environ.get("BASS_TRACE"))
    if trace:
        try:
            import antenv.axon_hooks  # noqa: F401
        except ImportError:
            # no NTFF shim installed (see test.py) -> tracing would crash
            os.environ["BASS_NEVER_TRACE"] = "1"
            trace = False
    res = run_bass_kernel_spmd(nc, in_maps, core_ids=list(range(NCORES)),
                               trace=trace)
    LAST["res"] = res

    out = np.empty((N_TOTAL, D), np.float32)
    for c in range(NCORES):
        out[c * NPC:(c + 1) * NPC] = res.results[c]["outT"].T.astype(np.float32)
    return out, out


# revision 20
# speedup vs baseline: 1.0843x; 1.0174x over previous
"""Trainium2 Bass kernel for NeuralODEMemory (nn_NeuralODEMemory_28355374088720).

Math (reference):
    dt = 0.1, 10 Euler steps over h (N=65536 rows, D=512):
        z = [h, t]                              # time feature column
        deriv = tanh(tanh(z @ W1.T + b1) @ W2.T + b2)
        h <- h + dt * deriv
    gate  = sigmoid([x, h10] @ Wg.T + bg)
    out   = gate * h10 + (1 - gate) * x

Kernel strategy (v2 — fp8 ODE loop, bf16 gate):
  * Data-parallel over 8 NeuronCores (8192 rows each); weights replicated.
  * Feature-major ("transposed") activation layout [D, rows] on chip: weights
    are the stationary operand, activations stream, each layer's PSUM output
    is already in the layout the next layer consumes.
  * ODE-loop matmuls (L1, L2) run in float8-e4m3 with DoubleRow (K=256/pass,
    ~1.77x the bf16 MM stream).  Numerical design validated by host-side
    simulation (rel err ~7e-3 vs the 2e-2 gate):
      - h accumulator stays fp32 (bf16 h pins max-err at 2.6e-2).
      - gate matmul stays bf16 (fp8 gate is the dominant max-err term:
        2.4e-2 alone; bf16 gate keeps the whole pipeline under 1e-2).
      - per-step *dither*: NPHASE fp8 copies of W1/W2 scaled by (1+eps_p);
        the ACT input scale undoes (1+eps_p) exactly.  This decorrelates
        the step-to-step weight quantization error (fixed e4m3 weight error
        otherwise integrates linearly through the 10 Euler steps).
      - the fp8 h-mirror is refreshed only every MP steps: the mirror lag
        (<= dt*|deriv| = 0.1) is below e4m3 quantization noise of h, and it
        halves the DVE cast load.
      - TRN e4m3 max normal is +-240 (not OCP's 448): all fp8 casts are
        host-side and clipped to +-240; on-chip casts only see tanh outputs
        in [-1,1] and h (|h|<7).
  * The time-feature column is folded into a per-step bias:
    z @ W1.T = h @ W1[:, :D].T + t * W1[:, D], so b1_eff(s) = b1 + t_s*W1[:,D].
    Bias-add and the fp8 scale-undo ride the ACT instruction (f(in*s + b)).
  * RBLK=2048 rows per block: PSUM tiles are [128, 2048] (4 banks, 2 bufs =
    all 8 banks) so each ACT instruction processes N=2048 — the ~460-cycle
    per-instruction ACT overhead made ACT the post-fp8 bottleneck at N=1024.
  * All elementwise work stays on DVE (GpSimd shares its SBUF port with
    DVE's 2-port perf mode — splitting buys little): h update is one fp32
    scalar_tensor_tensor per chunk, mirrors are plain casts, the gate
    combine runs at 16-bit 2x rate.
  * Host does the cheap prep: weight transpose/scale/cast, x/h transposes.
"""

import os
from contextlib import ExitStack

import numpy as np
import ml_dtypes

N_TOTAL = 65536
D = 512
NCORES = 8
NPC = N_TOTAL // NCORES          # rows per core
NUM_STEPS = 10
TIME_INTERVAL = 1.0
DT = TIME_INTERVAL / NUM_STEPS
P = 128
FK = D // P                      # feature chunks of 128 (4)
MMN = 512                        # matmul free dim (one PSUM bank)
RBLK = 2048                      # rows per block (4-bank PSUM groups)
NSUB = RBLK // MMN

NPHASE = 3                       # dithered fp8 weight copies
MP = 2                           # h-mirror refresh period (steps)
AW1 = 2.0 ** 12                  # fp8 scale on W1 (|W1|<=0.0442 -> <=189)
AW2 = 2.0 ** 12                  # fp8 scale on W2
# golden-ratio multiplicative dither phases, undone exactly in the ACT scale
EPS = [0.09 * ((p * 0.6180339887) % 1.0) - 0.045 for p in range(NPHASE)]

_CACHE = {}
LAST = {}                        # stash of the last run's BassKernelResults


def _build(npc):
    import concourse.mybir as mybir
    import concourse.tile as tile
    from concourse import bacc

    f32 = mybir.dt.float32
    bf16 = mybir.dt.bfloat16
    fp8 = mybir.dt.float8e4
    Act = mybir.ActivationFunctionType
    Alu = mybir.AluOpType
    DR = mybir.MatmulPerfMode.DoubleRow

    nblk = npc // RBLK

    nc = bacc.Bacc("TRN2", target_bir_lowering=False, debug=False,
                   num_devices=NCORES)

    hT = nc.dram_tensor("hT", [D, npc], f32, kind="ExternalInput").ap()
    xTb = nc.dram_tensor("xTb", [D, npc], bf16, kind="ExternalInput").ap()
    w1t = nc.dram_tensor("w1t", [NPHASE * D, D], fp8, kind="ExternalInput").ap()
    w2t = nc.dram_tensor("w2t", [NPHASE * D, D], fp8, kind="ExternalInput").ap()
    wgt = nc.dram_tensor("wgt", [2 * D, D], bf16, kind="ExternalInput").ap()
    b1e = nc.dram_tensor("b1e", [P, NUM_STEPS * FK], f32, kind="ExternalInput").ap()
    b2c = nc.dram_tensor("b2c", [P, FK], f32, kind="ExternalInput").ap()
    bgc = nc.dram_tensor("bgc", [P, FK], f32, kind="ExternalInput").ap()
    outT = nc.dram_tensor("outT", [D, npc], bf16, kind="ExternalOutput").ap()

    hTr = hT.rearrange("(k p) r -> p k r", p=P)
    xTr = xTb.rearrange("(k p) r -> p k r", p=P)
    outTr = outT.rearrange("(k p) r -> p k r", p=P)

    with tile.TileContext(nc) as tc:
        with ExitStack() as ctx:
            consts = ctx.enter_context(tc.tile_pool(name="consts", bufs=1))
            hpool = ctx.enter_context(tc.tile_pool(name="h", bufs=2))
            hbp = ctx.enter_context(tc.tile_pool(name="hb", bufs=2))
            apool = ctx.enter_context(tc.tile_pool(name="a", bufs=2))
            dpool = ctx.enter_context(tc.tile_pool(name="d", bufs=2))
            xbp = ctx.enter_context(tc.tile_pool(name="xb", bufs=1))
            hgp = ctx.enter_context(tc.tile_pool(name="hg", bufs=1))
            gp = ctx.enter_context(tc.tile_pool(name="g", bufs=2))
            scp = ctx.enter_context(tc.tile_pool(name="sc", bufs=2))
            otp = ctx.enter_context(tc.tile_pool(name="ot", bufs=2))
            psp = ctx.enter_context(tc.tile_pool(name="ps", bufs=2, space="PSUM"))

            # Replicated constants, resident for the whole kernel.
            w1 = consts.tile([P, NPHASE * FK, D], fp8)
            w2 = consts.tile([P, NPHASE * FK, D], fp8)
            wg = consts.tile([P, 2 * FK, D], bf16)
            b1 = consts.tile([P, NUM_STEPS * FK], f32)
            b2 = consts.tile([P, FK], f32)
            bg = consts.tile([P, FK], f32)

            def load_block(blk):
                rs = blk * RBLK
                h = hpool.tile([P, FK, RBLK], f32, tag="h")
                # per-chunk loads so the first mirror starts as soon as its
                # chunk lands
                for k in range(FK):
                    nc.sync.dma_start(h[:, k], hTr[:, k, rs:rs + RBLK])
                xb = xbp.tile([P, FK, RBLK], bf16, tag="xb")
                nc.sync.dma_start(xb[:], xTr[:, :, rs:rs + RBLK])
                return h, xb

            # DMA issue order follows first-use order: phase-0 weights and
            # the L1 bias are small and needed by the very first MM/ACT,
            # then block-0 activations, then the rest of the constants.
            w1r = w1t.rearrange("(f p) m -> p f m", p=P)
            w2r = w2t.rearrange("(f p) m -> p f m", p=P)
            nc.sync.dma_start(w1[:, 0:FK], w1r[:, 0:FK])
            nc.sync.dma_start(w2[:, 0:FK], w2r[:, 0:FK])
            nc.sync.dma_start(b1[:], b1e)
            nc.sync.dma_start(b2[:], b2c)
            h_cur, xb_cur = load_block(0)
            for ph in range(1, NPHASE):
                nc.sync.dma_start(w1[:, ph * FK:(ph + 1) * FK],
                                  w1r[:, ph * FK:(ph + 1) * FK])
                nc.sync.dma_start(w2[:, ph * FK:(ph + 1) * FK],
                                  w2r[:, ph * FK:(ph + 1) * FK])
            nc.sync.dma_start(wg[:], wgt.rearrange("(k p) m -> p k m", p=P))
            nc.sync.dma_start(bg[:], bgc)

            def mm_dr(ps_t, wtile, base, m, rhs):
                # one DR accumulation group: 2 k-pair groups x NSUB banks
                for kk in range(FK // 2):
                    for sub in range(NSUB):
                        nc.tensor.matmul(
                            ps_t[:, sub * MMN:(sub + 1) * MMN],
                            wtile[:, base + 2 * kk:base + 2 * kk + 2,
                                  m * P:(m + 1) * P],
                            rhs[:, 2 * kk:2 * kk + 2,
                                sub * MMN:(sub + 1) * MMN],
                            start=kk == 0, stop=kk == FK // 2 - 1,
                            perf_mode=DR)

            def mm_bf(ps_t, wk, m, rhs, rk, start, stop):
                for sub in range(NSUB):
                    nc.tensor.matmul(
                        ps_t[:, sub * MMN:(sub + 1) * MMN],
                        wg[:, wk, m * P:(m + 1) * P],
                        rhs[:, rk, sub * MMN:(sub + 1) * MMN],
                        start=start, stop=stop)

            for blk in range(nblk):
                rs = blk * RBLK
                h, xb = h_cur, xb_cur

                # mirror for step 0 (exact)
                hb = hbp.tile([P, FK, RBLK], fp8, tag="hb")
                for k in range(FK):
                    nc.vector.tensor_copy(hb[:, k], h[:, k])
                if blk + 1 < nblk:
                    h_cur, xb_cur = load_block(blk + 1)   # prefetch
                hg = None
                for s in range(NUM_STEPS):
                    ph = s % NPHASE
                    s1 = 1.0 / (AW1 * (1.0 + EPS[ph]))
                    s2 = 1.0 / (AW2 * (1.0 + EPS[ph]))
                    # layer 1: a = tanh(W1p.T-chunks @ hb + b1_eff(s))
                    a = apool.tile([P, FK, RBLK], fp8, tag="a")
                    for m in range(FK):
                        ps = psp.tile([P, RBLK], f32, tag="ps")
                        mm_dr(ps, w1, ph * FK, m, hb)
                        col = s * FK + m
                        nc.scalar.activation(a[:, m], ps[:], Act.Tanh,
                                             bias=b1[:, col:col + 1], scale=s1)
                    # layer 2: d = tanh(W2p.T-chunks @ a + b2)
                    d = dpool.tile([P, FK, RBLK], bf16, tag="d")
                    for m in range(FK):
                        ps = psp.tile([P, RBLK], f32, tag="ps")
                        mm_dr(ps, w2, ph * FK, m, a)
                        nc.scalar.activation(d[:, m], ps[:], Act.Tanh,
                                             bias=b2[:, m:m + 1], scale=s2)
                    if s < NUM_STEPS - 1:
                        # next step's mirror, cast BEFORE the update (lag-1):
                        # keeps the DVE STT/cast chain off the PE critical
                        # path (a lag-0 refresh stalls PE ~7.5us and lets HAM
                        # re-throttle).  The 1-step-stale mirror error
                        # (<= dt*|deriv|) is below e4m3 noise (sim: 1.07e-2).
                        hb = hbp.tile([P, FK, RBLK], fp8, tag="hb")
                        for k in range(FK):
                            nc.vector.tensor_copy(hb[:, k], h[:, k])
                        # h += dt * d  (fp32, in place)
                        for k in range(FK):
                            nc.vector.scalar_tensor_tensor(
                                h[:, k], d[:, k], float(DT), h[:, k],
                                Alu.mult, Alu.add)
                    else:
                        # final step: the update writes the bf16 gate operand
                        # directly (h10 is never needed in fp32), so gate
                        # h-side k-group k starts as soon as chunk k lands
                        hg = hgp.tile([P, FK, RBLK], bf16, tag="hg")
                        for k in range(FK):
                            nc.vector.scalar_tensor_tensor(
                                hg[:, k], d[:, k], float(DT), h[:, k],
                                Alu.mult, Alu.add)

                # gate (bf16): g = sigmoid([x, h10] @ Wg.T + bg)
                # h-side k-groups first, in STT completion order, so the
                # gate matmuls start before the final update chain finishes
                for m in range(FK):
                    ps = psp.tile([P, RBLK], f32, tag="ps")
                    for k in range(FK):
                        mm_bf(ps, FK + k, m, hg, k, k == 0, False)
                    for k in range(FK):
                        mm_bf(ps, k, m, xb, k, False, k == FK - 1)
                    g = gp.tile([P, RBLK], bf16, tag="g")
                    nc.scalar.activation(g[:], ps[:], Act.Sigmoid,
                                         bias=bg[:, m:m + 1], scale=1.0)
                    # out = x + g * (h10 - x), all 16-bit 2x-rate ops
                    dif = scp.tile([P, RBLK], bf16, tag="sc")
                    nc.vector.tensor_tensor(dif[:], hg[:, m], xb[:, m],
                                            Alu.subtract)
                    nc.vector.tensor_tensor(dif[:], g[:], dif[:], Alu.mult)
                    ot = otp.tile([P, RBLK], bf16, tag="ot")
                    nc.vector.tensor_tensor(ot[:], xb[:, m], dif[:], Alu.add)
                    nc.sync.dma_start(outTr[:, m, rs:rs + RBLK], ot[:])

    nc.compile()
    return nc


def _get_nc(npc):
    if npc not in _CACHE:
        _CACHE[npc] = _build(npc)
    return _CACHE[npc]


def _fp8_np():
    import concourse.mybir as mybir
    return mybir.dt.np(mybir.dt.float8e4)


def _host_prep(W1, b1, W2, b2, Wg, bg):
    E4 = _fp8_np()
    W1 = np.asarray(W1, np.float32)
    W2 = np.asarray(W2, np.float32)
    Wg = np.asarray(Wg, np.float32)
    b1 = np.asarray(b1, np.float32)
    b2 = np.asarray(b2, np.float32)
    bg = np.asarray(bg, np.float32)

    # NPHASE dithered fp8 copies, [in, out] layout, clipped to TRN e4m3 range
    w1t = np.concatenate(
        [np.clip(AW1 * (1.0 + e) * W1[:, :D], -240, 240).T for e in EPS],
        axis=0)
    w2t = np.concatenate(
        [np.clip(AW2 * (1.0 + e) * W2, -240, 240).T for e in EPS], axis=0)
    w1t = np.ascontiguousarray(w1t).astype(E4)
    w2t = np.ascontiguousarray(w2t).astype(E4)
    wgt = np.ascontiguousarray(
        np.concatenate([Wg[:, :D].T, Wg[:, D:].T], axis=0)
    ).astype(ml_dtypes.bfloat16)

    ts = (DT * np.arange(NUM_STEPS)).astype(np.float32)
    b1r = b1.reshape(FK, P)                                        # [m, p]
    wtr = np.ascontiguousarray(W1[:, D]).reshape(FK, P)            # [m, p]
    b1e = b1r[None, :, :] + ts[:, None, None] * wtr[None, :, :]    # [s, m, p]
    b1e = np.ascontiguousarray(b1e.transpose(2, 0, 1).reshape(P, NUM_STEPS * FK))
    b2c = np.ascontiguousarray(b2.reshape(FK, P).T)
    bgc = np.ascontiguousarray(bg.reshape(FK, P).T)
    return dict(w1t=w1t, w2t=w2t, wgt=wgt,
                b1e=b1e.astype(np.float32),
                b2c=b2c.astype(np.float32), bgc=bgc.astype(np.float32))


def kernel(current_node_features, previous_hidden_state, W1, b1, W2, b2, Wg, bg):
    from concourse.bass_utils import run_bass_kernel_spmd

    x = np.asarray(current_node_features, np.float32)
    h0 = np.asarray(previous_hidden_state, np.float32)
    weights = _host_prep(W1, b1, W2, b2, Wg, bg)

    in_maps = []
    for c in range(NCORES):
        sl = slice(c * NPC, (c + 1) * NPC)
        in_maps.append(dict(
            hT=np.ascontiguousarray(h0[sl].T),
            xTb=np.ascontiguousarray(x[sl].T).astype(ml_dtypes.bfloat16),
            **weights,
        ))

    nc = _get_nc(NPC)
    trace = bool(os.environ.get("BASS_TRACE"))
    if trace:
        try:
            import antenv.axon_hooks  # noqa: F401
        except ImportError:
            # no NTFF shim installed (see test.py) -> tracing would crash
            os.environ["BASS_NEVER_TRACE"] = "1"
            trace = False
    res = run_bass_kernel_spmd(nc, in_maps, core_ids=list(range(NCORES)),
                               trace=trace)
    LAST["res"] = res

    out = np.empty((N_TOTAL, D), np.float32)
    for c in range(NCORES):
        out[c * NPC:(c + 1) * NPC] = res.results[c]["outT"].T.astype(np.float32)
    return out, out
